# revision 6
# baseline (speedup 1.0000x reference)
"""GAT layer kernel for Trainium2, data-parallel over 8 NeuronCores.

Problem (per graph): X [1024, 128] f32, W [64, 128], a [1, 128]
  h = X @ W.T                       [1024, 64]
  s_src = h @ a[:64], s_dst = h @ a[64:]
  e[i,j] = leaky_relu(s_src[i] + s_dst[j], 0.01)
  att = softmax_j(e); out = att @ h  [1024, 64]

32 graphs total -> 4 per core across 8 cores (inputs W/a replicated).

Per-core kernel strategy (all tiles 128-partition):
  - Attention is built directly in TRANSPOSED tile layout
    PT[j, i] = exp(lrelu(s_src[i] + s_dst[j])), tiles [128 j x 1024 i],
    which is exactly the lhsT the TensorE needs for att @ h - no big
    transposes.  exp(lrelu(x)) = max(exp(x), exp(x/100)) since exp is
    monotonic, so no Lrelu activation needed:
        PT = max(A[i]*B[j], C[i]*D[j])
        A = exp(s_src), B = exp(s_dst), C = exp(s_src/100), D = exp(s_dst/100)
  - s_dst columns fall out of the h matmul as an extra rhs column.
  - s_src is computed as a [1, 1024] row (M=1 matmuls), replicated across
    partitions with a K=1 ones outer-product matmul into PSUM, and exp'd
    once per graph into replicated SBUF tiles A_rep / C_rep (bf16).
    Per j-tile the pass is then just tensor_scalar + scalar_tensor_tensor
    on the vector engine.
  - A ones column is appended to h so the accumulation matmul
    PT.T @ [h | 1] produces both h' and the softmax normalizer Z in PSUM;
    the epilogue multiplies by 1/Z per partition.
"""

import sys

if "/opt/trn_rl_repo" not in sys.path:
    sys.path.insert(0, "/opt/trn_rl_repo")

from contextlib import ExitStack

import numpy as np

import concourse.bass as bass
import concourse.mybir as mybir
import concourse.tile as tile
from concourse import bacc
from concourse.bass_utils import run_bass_kernel_spmd
from concourse.masks import make_identity

# ---- hardcoded problem shapes -------------------------------------------
N_TOTAL = 32          # graphs
N_CORES = 8
N_PER = N_TOTAL // N_CORES   # 4 graphs per core
V = 1024              # nodes per graph
F = 128               # input features
H = 64                # hidden features
NT = V // 128         # 8 tiles of 128 nodes
SLOPE = 0.01          # leaky_relu negative slope

FP32 = mybir.dt.float32
BF16 = mybir.dt.bfloat16
AF = mybir.ActivationFunctionType
OP = mybir.AluOpType


def build_gat_program():
    """Build the per-core Bass program (same program on all 8 cores)."""
    nc = bacc.Bacc("TRN2", target_bir_lowering=False, debug=False)

    feat_d = nc.dram_tensor("features", [N_PER, V, F], FP32, kind="ExternalInput")
    w_d = nc.dram_tensor("W", [H, F], FP32, kind="ExternalInput")
    a_d = nc.dram_tensor("a", [1, 2 * H], FP32, kind="ExternalInput")
    out_d = nc.dram_tensor("out", [N_PER, V, H], FP32, kind="ExternalOutput")

    feat = feat_d.ap()
    out = out_d.ap()

    with tile.TileContext(nc) as tc, ExitStack() as ctx:
        # ---- pools -------------------------------------------------------
        consts = ctx.enter_context(tc.tile_pool(name="consts", bufs=1))
        xpool = ctx.enter_context(tc.tile_pool(name="x", bufs=3))
        xtpool = ctx.enter_context(tc.tile_pool(name="xt", bufs=2 * NT))
        augpool = ctx.enter_context(tc.tile_pool(name="aug", bufs=2 * NT))
        sdpool = ctx.enter_context(tc.tile_pool(name="sd", bufs=2))
        srpool = ctx.enter_context(tc.tile_pool(name="srow", bufs=2))
        reppool = ctx.enter_context(tc.tile_pool(name="rep", bufs=2))
        t2pool = ctx.enter_context(tc.tile_pool(name="t2", bufs=2))
        ppool = ctx.enter_context(tc.tile_pool(name="p", bufs=2 * NT))
        rzpool = ctx.enter_context(tc.tile_pool(name="rz", bufs=2))
        opool = ctx.enter_context(tc.tile_pool(name="o", bufs=3))

        # PSUM bank budget (8 total): ps_t(shared tags)=2, ps_h=2, ps_srep=2, ps_out=2
        ps_t = ctx.enter_context(tc.tile_pool(name="ps_t", bufs=2, space="PSUM"))
        ps_h = ctx.enter_context(tc.tile_pool(name="ps_h", bufs=2, space="PSUM"))
        ps_srep = ctx.enter_context(tc.tile_pool(name="ps_srep", bufs=1, space="PSUM"))
        ps_out = ctx.enter_context(tc.tile_pool(name="ps_out", bufs=2, space="PSUM"))

        # ---- constants / weight prep ------------------------------------
        ident = consts.tile([128, 128], FP32)
        make_identity(nc, ident[:])

        ones_row = consts.tile([1, 128], FP32)
        nc.vector.memset(ones_row[:], 1.0)

        a_sb = consts.tile([1, 2 * H], FP32)
        nc.sync.dma_start(a_sb[:], a_d.ap()[:])
        w_sb = consts.tile([H, F], FP32)
        nc.sync.dma_start(w_sb[:], w_d.ap()[:])
        wb = consts.tile([H, F], BF16)
        nc.vector.tensor_copy(wb[:], w_sb[:])

        # a halves -> bf16 columns [H, 2] (via PE transpose of the row)
        asrc_ps = ps_t.tile([H, 1], FP32, tag="xt")
        nc.tensor.transpose(asrc_ps[:], a_sb[0:1, 0:H], ident[0:1, 0:1])
        adst_ps = ps_t.tile([H, 1], FP32, tag="xt")
        nc.tensor.transpose(adst_ps[:], a_sb[0:1, H : 2 * H], ident[0:1, 0:1])
        a2 = consts.tile([H, 2], BF16)
        nc.vector.tensor_copy(a2[:, 0:1], asrc_ps[:])
        nc.vector.tensor_copy(a2[:, 1:2], adst_ps[:])

        # w_src/w_dst = W.T @ a_halves : [F, 2] (f32 psum)
        wcols_ps = ps_t.tile([F, 2], FP32, tag="xt")
        nc.tensor.matmul(wcols_ps[:], lhsT=wb[:], rhs=a2[:], start=True, stop=True)
        wsrc_sb = consts.tile([F, 1], BF16)
        nc.vector.tensor_copy(wsrc_sb[:], wcols_ps[:, 0:1])

        # rhs_w = [w_dst | W.T] : [F, 1+H] bf16
        wt_ps = ps_t.tile([F, H], FP32, tag="xt")
        nc.tensor.transpose(wt_ps[:], w_sb[:], ident[0:H, 0:H])
        rhs_w = consts.tile([F, 1 + H], BF16)
        nc.vector.tensor_copy(rhs_w[:, 0:1], wcols_ps[:, 1:2])
        nc.vector.tensor_copy(rhs_w[:, 1 : 1 + H], wt_ps[:])

        # ---- per-graph pipeline -----------------------------------------
        for g in range(N_PER):
            # -- loop A: load X tiles, transpose, h-matmul, score pieces --
            sdst_g = sdpool.tile([128, NT], FP32)
            augs = []
            xtbs = []
            for jt in range(NT):
                x_t = xpool.tile([128, F], FP32)
                nc.sync.dma_start(x_t[:], feat[g, jt * 128 : (jt + 1) * 128, :])

                xt_ps = ps_t.tile([128, 128], FP32, tag="xt")
                nc.tensor.transpose(xt_ps[:], x_t[:], ident[:])
                xtb = xtpool.tile([128, 128], BF16)
                nc.scalar.copy(xtb[:], xt_ps[:])

                # [s_dst | h] for this node tile
                h_ps = ps_h.tile([128, 1 + H], FP32)
                nc.tensor.matmul(h_ps[:], lhsT=xtb[:], rhs=rhs_w[:], start=True, stop=True)

                aug = augpool.tile([128, 2 + H], BF16)
                nc.gpsimd.memset(aug[:, 1 + H : 2 + H], 1.0)
                nc.scalar.copy(aug[:, 0 : 1 + H], h_ps[:])
                augs.append(aug)
                xtbs.append(xtb)

                nc.scalar.copy(sdst_g[:, jt : jt + 1], h_ps[:, 0:1])

            # -- A2: s_src row segments, replicate across partitions, exp --
            srow_sb = srpool.tile([1, V], FP32)
            for half in range(2):
                srow_ps = ps_t.tile([1, 512], FP32, tag="xt", name=f"srow_{g}_{half}")
                for q in range(4):
                    jt = half * 4 + q
                    nc.tensor.matmul(
                        srow_ps[0:1, q * 128 : (q + 1) * 128],
                        lhsT=wsrc_sb[:],
                        rhs=xtbs[jt][:],
                        start=True,
                        stop=True,
                    )
                nc.scalar.copy(srow_sb[0:1, half * 512 : (half + 1) * 512], srow_ps[:])

            srep_ps = ps_srep.tile([128, V], FP32)
            nc.tensor.matmul(
                srep_ps[:, 0:512], lhsT=ones_row[:], rhs=srow_sb[0:1, 0:512],
                start=True, stop=True,
            )
            nc.tensor.matmul(
                srep_ps[:, 512:1024], lhsT=ones_row[:], rhs=srow_sb[0:1, 512:1024],
                start=True, stop=True,
            )

            a_rep = reppool.tile([128, V], BF16, tag="a_rep")
            nc.scalar.activation(a_rep[:], srep_ps[:], AF.Exp)
            c_rep = reppool.tile([128, V], BF16, tag="c_rep")
            nc.scalar.activation(c_rep[:], srep_ps[:], AF.Exp, scale=SLOPE)

            b_g = sdpool.tile([128, NT], FP32, tag="b_g")
            nc.scalar.activation(b_g[:], sdst_g[:], AF.Exp)
            d_g = sdpool.tile([128, NT], FP32, tag="d_g")
            nc.scalar.activation(d_g[:], sdst_g[:], AF.Exp, scale=SLOPE)

            # -- loop B: attention tiles, then accumulation matmuls -------
            # (each PSUM accumulation group runs start->stop back-to-back so
            #  no two pending groups share a bank)
            po = [ps_out.tile([128, 4 * (H + 1)], FP32, name=f"po_{g}_{i}", tag="po") for i in range(2)]
            p_ts = []
            for jt in range(NT):
                t2 = t2pool.tile([128, V], BF16)
                nc.vector.tensor_scalar(
                    t2[:], c_rep[:], d_g[:, jt : jt + 1], None, OP.mult
                )
                p_t = ppool.tile([128, V], BF16)
                nc.vector.scalar_tensor_tensor(
                    p_t[:], in0=a_rep[:], scalar=b_g[:, jt : jt + 1], in1=t2[:],
                    op0=OP.mult, op1=OP.max,
                )
                p_ts.append(p_t)
            for it in range(NT):
                t, r = it // 4, it % 4
                for jt in range(NT):
                    nc.tensor.matmul(
                        po[t][:, r * (H + 1) : (r + 1) * (H + 1)],
                        lhsT=p_ts[jt][:, it * 128 : (it + 1) * 128],
                        rhs=augs[jt][:, 1 : 2 + H],
                        start=(jt == 0),
                        stop=(jt == NT - 1),
                    )

            # -- loop C: normalize + store --------------------------------
            for it in range(NT):
                t, r = it // 4, it % 4
                base = r * (H + 1)
                rz = rzpool.tile([128, 1], FP32)
                nc.vector.reciprocal(rz[:], po[t][:, base + H : base + H + 1])
                o_sb = opool.tile([128, H], FP32)
                nc.vector.tensor_scalar(
                    o_sb[:], po[t][:, base : base + H], rz[:], None, OP.mult
                )
                nc.sync.dma_start(out[g, it * 128 : (it + 1) * 128, :], o_sb[:])

    nc.compile()
    return nc


_NC_CACHE = None


def _get_program():
    global _NC_CACHE
    if _NC_CACHE is None:
        _NC_CACHE = build_gat_program()
    return _NC_CACHE


def kernel(features: np.ndarray, W: np.ndarray, a: np.ndarray) -> np.ndarray:
    """Full-input entry point: features [32, 1024, 128], W [64, 128], a [1, 128]."""
    assert features.shape == (N_TOTAL, V, F)
    nc = _get_program()

    features = np.ascontiguousarray(features, dtype=np.float32)
    W = np.ascontiguousarray(W, dtype=np.float32)
    a = np.ascontiguousarray(a, dtype=np.float32)

    in_maps = [
        {
            "features": features[c * N_PER : (c + 1) * N_PER],
            "W": W,
            "a": a,
        }
        for c in range(N_CORES)
    ]
    res = run_bass_kernel_spmd(nc, in_maps, core_ids=list(range(N_CORES)))
    outs = [res.results[c]["out"] for c in range(N_CORES)]
    return np.concatenate(outs, axis=0)


if __name__ == "__main__":
    # smoke-test build only
    prog = build_gat_program()
    print("program built:", len(prog.m.functions[0].basic_blocks[0].instructions)
          if hasattr(prog.m.functions[0], "basic_blocks") else "ok")
